# revision 24
# baseline (speedup 1.0000x reference)
"""Bilinear field-interaction kernel for Trainium2 (8 NeuronCores, SPMD).

Computes out[b, p, :] = (v_i @ W_p) * v_j for all P = 496 field pairs
(i < j) of NF = 32 fields, D = 64, batch 2048, f32.

Strategy (data-parallel over batch, W replicated):
  - Each core gets a 256-row batch slice (2 blocks of 128 partitions).
  - Per block, per field i: transpose V_i -> viT [64d, 128b] on the PE
    (stationary matmul operand).
  - W resident in SBUF as [64d, (pair, e)] so 8 pairs' W stream as one
    [64, 512] moving operand: psum[b, (pair e)] = viT.T @ W_chunk.
  - DVE multiplies psum by the v_j slice (consecutive j for fixed i ->
    contiguous columns of the feat tile) straight into the out tile.
  - Out tiles cover both blocks -> ~0.5 MB contiguous-chunk DMA stores.
"""

import os

import numpy as np

NF = 32
D = 64
NPAIR = NF * (NF - 1) // 2  # 496
B_TOTAL = 2048
NCORES = 8
B_CORE = B_TOTAL // NCORES  # 256
P = 128
NBLK = B_CORE // P  # 2
CHUNK = 8  # pairs per matmul (N = CHUNK*D = 512 columns, one PSUM bank)
STORE_PAIRS = 32  # pairs per output store (x2 blocks = 2 MB)
WSLAB = 62  # pairs per W-load DMA (~1 MB each)

LAST_EXEC_NS = None

_BUILT = {}


def _pair_base(i):
    # index of pair (i, i+1) in itertools.combinations(range(NF), 2) order
    return i * (NF - 1) - i * (i - 1) // 2


def _build_bass(iters=1):
    import concourse.bass as bass
    import concourse.mybir as mybir
    import concourse.tile as tile
    from concourse import bacc
    from concourse.masks import make_identity

    f32 = mybir.dt.float32

    nc = bacc.Bacc(
        "TRN2",
        target_bir_lowering=False,
        debug=False,
        enable_asserts=False,
        num_devices=NCORES,
    )
    feat = nc.dram_tensor(
        "feature_emb", [B_CORE, NF, D], f32, kind="ExternalInput"
    ).ap()
    W = nc.dram_tensor("bilinear_W", [NPAIR, D, D], f32, kind="ExternalInput").ap()
    out = nc.dram_tensor("out", [B_CORE, NPAIR, D], f32, kind="ExternalOutput").ap()

    # out viewed as [b_in_block, blk, (pair*D)] for stores
    out_v = out.rearrange("(blk b) p e -> b blk (p e)", blk=NBLK)

    with tile.TileContext(nc) as tc:
        with (
            tc.tile_pool(name="consts", bufs=1) as consts,
            tc.tile_pool(name="wpool", bufs=1) as wpool,
            tc.tile_pool(name="featp", bufs=2) as featp,
            tc.tile_pool(name="vitp", bufs=3) as vitp,
            tc.tile_pool(name="outp", bufs=3) as outp,
            tc.tile_pool(name="mmps", bufs=3, space="PSUM") as mmps,
            tc.tile_pool(name="trps", bufs=2, space="PSUM") as trps,
        ):
            ident = consts.tile([P, P], f32)
            make_identity(nc, ident)

            for _ in range(iters):
                # feature tiles, one per 128-row block, column-duplicated:
                # [128, (f, two, d)]. Slot two=0 is DMA'd from DRAM, slot
                # two=1 is a GPSIMD copy. The [128, 128] per-field slices
                # transpose into dual-half viT in one shot; muls read v_j
                # from slot 0.
                feat_dup = []
                for blk in range(NBLK):
                    td = featp.tile([P, NF * 2 * D], f32, tag=f"featd{blk}")
                    td_v = td.rearrange("p (f two d) -> p f two d", two=2, d=D)
                    nc.scalar.dma_start(
                        out=td_v[:, :, 0, :],
                        in_=feat[blk * P : (blk + 1) * P],
                    )
                    nc.gpsimd.tensor_copy(out=td_v[:, :, 1, :], in_=td_v[:, :, 0, :])
                    feat_dup.append(td)

                # W resident in SBUF, even/odd layout: partitions 0:64 hold
                # even pairs' [d, e] blocks, 64:128 odd pairs; free dim is
                # (t, e) with t = pair//2. 128-partition DMAs use all 16
                # SBUF ports (the 64-partition strided layout is half-BW).
                w_sb = wpool.tile([P, (NPAIR // 2) * D], f32, tag="w")
                for p0 in range(0, NPAIR, WSLAB):
                    w = min(WSLAB, NPAIR - p0)
                    t0 = p0 // 2
                    nc.scalar.dma_start(
                        out=w_sb[:, t0 * D : (t0 + w // 2) * D],
                        in_=W[p0 : p0 + w].rearrange(
                            "(t two) d e -> (two d) t e", two=2
                        ),
                    )

                for i in range(NF - 1):
                    m = NF - 1 - i  # pairs in this i-group
                    base = _pair_base(i)

                    # transpose duplicated V_i per block: [128 b, (2 x 64 d)]
                    # -> [128 (two, d), 128 b], so the matmul lhsT exists at
                    # base partitions 0 and 64 (matching even/odd rhs rows)
                    vts = []
                    for blk in range(NBLK):
                        tp = trps.tile([P, P], f32, tag="tp")
                        nc.tensor.transpose(
                            tp, feat_dup[blk][:, i * 2 * D : (i + 1) * 2 * D], ident
                        )
                        vt = vitp.tile([P, P], f32, tag=f"vt{blk}")
                        nc.scalar.copy(out=vt, in_=tp)
                        vts.append(vt)

                    # store granule: STORE_PAIRS pairs x both blocks (~2 MB).
                    # Per parity, matmuls fill a 2-bank psum tile in
                    # N=512 sub-chunks; ONE DVE mul then covers the whole
                    # parity run (fewer ops -> less per-op overhead).
                    for s0 in range(0, m, STORE_PAIRS):
                        sn = min(STORE_PAIRS, m - s0)
                        gp0 = base + s0
                        ot = outp.tile([P, NBLK, STORE_PAIRS * D], f32, tag="ot")
                        ot_v = ot.rearrange(
                            "p b (q two e) -> p b two q e", two=2, e=D
                        )
                        for pi in (0, 1):
                            plist = [
                                p for p in range(gp0, gp0 + sn) if p % 2 == pi
                            ]
                            if not plist:
                                continue
                            cp = len(plist)  # <= STORE_PAIRS // 2
                            t0 = plist[0] // 2
                            j0 = i + 1 + (plist[0] - base)
                            sig = (plist[0] - gp0) % 2  # slot parity in ot
                            q0 = (plist[0] - gp0) // 2
                            for blk in range(NBLK):
                                ps = mmps.tile([P, 2 * CHUNK * D], f32, tag="ps")
                                for ch0 in range(0, cp, CHUNK):
                                    c = min(CHUNK, cp - ch0)
                                    nc.tensor.matmul(
                                        ps[:, ch0 * D : (ch0 + c) * D],
                                        vts[blk][pi * D : (pi + 1) * D, :],
                                        w_sb[
                                            pi * D : (pi + 1) * D,
                                            (t0 + ch0) * D : (t0 + ch0 + c) * D,
                                        ],
                                        start=True,
                                        stop=True,
                                    )
                                nc.vector.tensor_mul(
                                    ot_v[:, blk, sig, q0 : q0 + cp, :],
                                    ps.rearrange("p (q e) -> p q e", e=D)[
                                        :, :cp, :
                                    ],
                                    feat_dup[blk].rearrange(
                                        "p (g two c) -> p two g c", two=2, c=2 * D
                                    )[:, j0 % 2, j0 // 2 : j0 // 2 + cp, 0:D],
                                )
                        nc.sync.dma_start(
                            out=out_v[:, :, gp0 * D : (gp0 + sn) * D],
                            in_=ot[:, :, : sn * D],
                        )

    nc.compile()
    return nc


def _get_nc(iters=1):
    if iters not in _BUILT:
        _BUILT[iters] = _build_bass(iters)
    return _BUILT[iters]


class PjrtRunner:
    """Reusable jitted runner for a prebuilt Bass module on 8 cores.

    Unlike run_bass_kernel_spmd, keeps the jitted fn + device-resident
    inputs alive so repeated calls don't recompile or re-transfer, letting
    wall-clock deltas measure on-device execution time.
    """

    def __init__(self, nc, unroll=1):
        import jax
        import concourse.mybir as mybir
        from concourse import bass2jax

        bass2jax.install_neuronx_cc_hook()
        self.nc = nc
        partition_name = (
            nc.partition_id_tensor.name if nc.partition_id_tensor else None
        )
        in_names, out_names, out_avals = [], [], []
        self.out_shapes = []
        for alloc in nc.m.functions[0].allocations:
            if not isinstance(alloc, mybir.MemoryLocationSet):
                continue
            name = alloc.memorylocations[0].name
            if alloc.kind == "ExternalInput":
                if name != partition_name:
                    in_names.append(name)
            elif alloc.kind == "ExternalOutput":
                shape = tuple(alloc.tensor_shape)
                dtype = mybir.dt.np(alloc.dtype)
                out_names.append(name)
                out_avals.append(jax.core.ShapedArray(shape, dtype))
                self.out_shapes.append((shape, dtype))
        self.in_names = in_names
        self.out_names = out_names
        bind_names = list(in_names + out_names)
        if partition_name is not None:
            bind_names.append(partition_name)
        bind_names = tuple(bind_names)

        n_in = len(in_names)

        def _body(*args):
            operands = list(args)
            if partition_name is not None:
                operands.append(bass2jax.partition_id_tensor())
            # repeated binds: BassEffect is an ordered effect, so launches
            # serialize and aren't CSE'd despite identical operands
            for _ in range(unroll):
                outs = bass2jax._bass_exec_p.bind(
                    *operands,
                    out_avals=tuple(out_avals),
                    in_names=bind_names,
                    out_names=tuple(out_names),
                    lowering_input_output_aliases=(),
                    sim_require_finite=False,
                    sim_require_nnan=False,
                    nc=nc,
                )
            return tuple(outs)

        from jax.sharding import Mesh, NamedSharding, PartitionSpec
        from jax.experimental.shard_map import shard_map

        devices = jax.devices()[:NCORES]
        self.mesh = Mesh(np.asarray(devices), ("core",))
        self.sharding = NamedSharding(self.mesh, PartitionSpec("core"))
        n_args = len(in_names) + len(out_names)
        self.fn = jax.jit(
            shard_map(
                _body,
                mesh=self.mesh,
                in_specs=(PartitionSpec("core"),) * n_args,
                out_specs=(PartitionSpec("core"),) * len(out_names),
                check_rep=False,
            ),
            keep_unused=True,
        )
        self.args = None

    def set_inputs(self, in_maps):
        import jax

        per_core = [[np.asarray(m[n]) for n in self.in_names] for m in in_maps]
        arrs = [
            np.concatenate([per_core[c][i] for c in range(NCORES)], axis=0)
            for i in range(len(self.in_names))
        ]
        for shape, dtype in self.out_shapes:
            arrs.append(np.zeros((NCORES * shape[0],) + shape[1:], dtype))
        self.args = [jax.device_put(a, self.sharding) for a in arrs]

    def run(self):
        import jax

        outs = self.fn(*self.args)
        jax.block_until_ready(outs)
        return outs


def make_in_maps(feature_emb: np.ndarray, bilinear_W: np.ndarray):
    feature_emb = np.ascontiguousarray(feature_emb, dtype=np.float32)
    bilinear_W = np.ascontiguousarray(bilinear_W, dtype=np.float32)
    assert feature_emb.shape == (B_TOTAL, NF, D)
    assert bilinear_W.shape == (NPAIR, D, D)
    return [
        {
            "feature_emb": feature_emb[c * B_CORE : (c + 1) * B_CORE],
            "bilinear_W": bilinear_W,
        }
        for c in range(NCORES)
    ]


def kernel(feature_emb: np.ndarray, bilinear_W: np.ndarray) -> np.ndarray:
    from concourse.bass_utils import run_bass_kernel_spmd

    in_maps = make_in_maps(feature_emb, bilinear_W)
    nc = _get_nc()
    res = run_bass_kernel_spmd(nc, in_maps, core_ids=list(range(NCORES)))
    return np.concatenate([r["out"] for r in res.results], axis=0)


# revision 27
# speedup vs baseline: 1.5356x; 1.5356x over previous
"""Bilinear field-interaction kernel for Trainium2 (8 NeuronCores, SPMD).

Computes out[b, p, :] = (v_i @ W_p) * v_j for all 496 field pairs
(i < j) of NF = 32 fields, D = 64, batch 2048, f32.

Strategy (data-parallel over batch, W replicated on every core):
  - Each core gets a 256-row batch slice (2 blocks of 128 partitions).
  - W resident in SBUF in an even/odd layout: [128 (parity, d), (t, e)]
    with t = pair//2. Each pair's natural [d, e] block lands in one
    partition half, so the load DMA spans all 128 partitions (full
    16-port SBUF bandwidth; a 64-partition layout is half-rate).
  - Per block, per field i: one PE transpose of a column-duplicated
    [128 b, (2 x 64 d)] feat slice yields viT stacked in both partition
    halves, matching the even/odd rhs row groups.
  - Matmuls per parity run: psum[128 b, c*64] = viT.T @ W[t-run], with
    N up to 512 (8 same-parity pairs, one PSUM bank).
  - DVE multiplies psum by the v_j slice (stride-2 field runs of the
    feat tile) straight into interleaved slots of the out tile.
  - Out tiles cover 16 pairs x both blocks -> 1 MB stores with 4 KB
    contiguous runs per partition; 6-deep out-tile pool keeps the
    store queue fed.
"""

import os

import numpy as np

NF = 32
D = 64
NPAIR = NF * (NF - 1) // 2  # 496
B_TOTAL = 2048
NCORES = 8
B_CORE = B_TOTAL // NCORES  # 256
P = 128
NBLK = B_CORE // P  # 2
CHUNK = 8  # pairs per matmul (N = CHUNK*D = 512 columns, one PSUM bank)
STORE_PAIRS = 16  # pairs per output store (x2 blocks = 1 MB)
WSLAB = 62  # pairs per W-load DMA (~1 MB each)

LAST_EXEC_NS = None

_BUILT = {}


def _pair_base(i):
    # index of pair (i, i+1) in itertools.combinations(range(NF), 2) order
    return i * (NF - 1) - i * (i - 1) // 2


def _build_bass(iters=1):
    import concourse.bass as bass
    import concourse.mybir as mybir
    import concourse.tile as tile
    from concourse import bacc
    from concourse.masks import make_identity

    f32 = mybir.dt.float32

    nc = bacc.Bacc(
        "TRN2",
        target_bir_lowering=False,
        debug=False,
        enable_asserts=False,
        num_devices=NCORES,
    )
    feat = nc.dram_tensor(
        "feature_emb", [B_CORE, NF, D], f32, kind="ExternalInput"
    ).ap()
    W = nc.dram_tensor("bilinear_W", [NPAIR, D, D], f32, kind="ExternalInput").ap()
    out = nc.dram_tensor("out", [B_CORE, NPAIR, D], f32, kind="ExternalOutput").ap()

    # out viewed as [b_in_block, blk, (pair*D)] for stores
    out_v = out.rearrange("(blk b) p e -> b blk (p e)", blk=NBLK)

    with tile.TileContext(nc) as tc:
        with (
            tc.tile_pool(name="consts", bufs=1) as consts,
            tc.tile_pool(name="wpool", bufs=1) as wpool,
            tc.tile_pool(name="featp", bufs=2) as featp,
            tc.tile_pool(name="vitp", bufs=3) as vitp,
            tc.tile_pool(name="outp", bufs=6) as outp,
            tc.tile_pool(name="mmps", bufs=6, space="PSUM") as mmps,
            tc.tile_pool(name="trps", bufs=2, space="PSUM") as trps,
        ):
            ident = consts.tile([P, P], f32)
            make_identity(nc, ident)

            for _ in range(iters):
                # feature tiles, one per 128-row block, column-duplicated:
                # [128, (f, two, d)]. Slot two=0 is DMA'd from DRAM, slot
                # two=1 is a GPSIMD copy. The [128, 128] per-field slices
                # transpose into dual-half viT in one shot; muls read v_j
                # from slot 0.
                feat_dup = []
                for blk in range(NBLK):
                    td = featp.tile([P, NF * 2 * D], f32, tag=f"featd{blk}")
                    td_v = td.rearrange("p (f two d) -> p f two d", two=2, d=D)
                    nc.scalar.dma_start(
                        out=td_v[:, :, 0, :],
                        in_=feat[blk * P : (blk + 1) * P],
                    )
                    nc.gpsimd.tensor_copy(out=td_v[:, :, 1, :], in_=td_v[:, :, 0, :])
                    feat_dup.append(td)

                # W resident in SBUF, even/odd layout: partitions 0:64 hold
                # even pairs' [d, e] blocks, 64:128 odd pairs; free dim is
                # (t, e) with t = pair//2. 128-partition DMAs use all 16
                # SBUF ports (the 64-partition strided layout is half-BW).
                w_sb = wpool.tile([P, (NPAIR // 2) * D], f32, tag="w")
                for p0 in range(0, NPAIR, WSLAB):
                    w = min(WSLAB, NPAIR - p0)
                    t0 = p0 // 2
                    nc.scalar.dma_start(
                        out=w_sb[:, t0 * D : (t0 + w // 2) * D],
                        in_=W[p0 : p0 + w].rearrange(
                            "(t two) d e -> (two d) t e", two=2
                        ),
                    )

                for i in range(NF - 1):
                    m = NF - 1 - i  # pairs in this i-group
                    base = _pair_base(i)

                    # transpose duplicated V_i per block: [128 b, (2 x 64 d)]
                    # -> [128 (two, d), 128 b], so the matmul lhsT exists at
                    # base partitions 0 and 64 (matching even/odd rhs rows)
                    vts = []
                    for blk in range(NBLK):
                        tp = trps.tile([P, P], f32, tag="tp")
                        nc.tensor.transpose(
                            tp, feat_dup[blk][:, i * 2 * D : (i + 1) * 2 * D], ident
                        )
                        vt = vitp.tile([P, P], f32, tag=f"vt{blk}")
                        nc.scalar.copy(out=vt, in_=tp)
                        vts.append(vt)

                    # store granule: STORE_PAIRS pairs x both blocks (~2 MB).
                    # Per parity, matmuls fill a 2-bank psum tile in
                    # N=512 sub-chunks; ONE DVE mul then covers the whole
                    # parity run (fewer ops -> less per-op overhead).
                    for s0 in range(0, m, STORE_PAIRS):
                        sn = min(STORE_PAIRS, m - s0)
                        gp0 = base + s0
                        ot = outp.tile([P, NBLK, STORE_PAIRS * D], f32, tag="ot")
                        ot_v = ot.rearrange(
                            "p b (q two e) -> p b two q e", two=2, e=D
                        )
                        for pi in (0, 1):
                            plist = [
                                p for p in range(gp0, gp0 + sn) if p % 2 == pi
                            ]
                            if not plist:
                                continue
                            cp = len(plist)  # <= STORE_PAIRS // 2
                            t0 = plist[0] // 2
                            j0 = i + 1 + (plist[0] - base)
                            sig = (plist[0] - gp0) % 2  # slot parity in ot
                            q0 = (plist[0] - gp0) // 2
                            for blk in range(NBLK):
                                ps = mmps.tile([P, CHUNK * D], f32, tag="ps")
                                for ch0 in range(0, cp, CHUNK):
                                    c = min(CHUNK, cp - ch0)
                                    nc.tensor.matmul(
                                        ps[:, ch0 * D : (ch0 + c) * D],
                                        vts[blk][pi * D : (pi + 1) * D, :],
                                        w_sb[
                                            pi * D : (pi + 1) * D,
                                            (t0 + ch0) * D : (t0 + ch0 + c) * D,
                                        ],
                                        start=True,
                                        stop=True,
                                    )
                                nc.vector.tensor_mul(
                                    ot_v[:, blk, sig, q0 : q0 + cp, :],
                                    ps.rearrange("p (q e) -> p q e", e=D)[
                                        :, :cp, :
                                    ],
                                    feat_dup[blk].rearrange(
                                        "p (g two c) -> p two g c", two=2, c=2 * D
                                    )[:, j0 % 2, j0 // 2 : j0 // 2 + cp, 0:D],
                                )
                        nc.sync.dma_start(
                            out=out_v[:, :, gp0 * D : (gp0 + sn) * D],
                            in_=ot[:, :, : sn * D],
                        )

    nc.compile()
    return nc


def _get_nc(iters=1):
    if iters not in _BUILT:
        _BUILT[iters] = _build_bass(iters)
    return _BUILT[iters]


class PjrtRunner:
    """Reusable jitted runner for a prebuilt Bass module on 8 cores.

    Unlike run_bass_kernel_spmd, keeps the jitted fn + device-resident
    inputs alive so repeated calls don't recompile or re-transfer, letting
    wall-clock deltas measure on-device execution time.
    """

    def __init__(self, nc, unroll=1):
        import jax
        import concourse.mybir as mybir
        from concourse import bass2jax

        bass2jax.install_neuronx_cc_hook()
        self.nc = nc
        partition_name = (
            nc.partition_id_tensor.name if nc.partition_id_tensor else None
        )
        in_names, out_names, out_avals = [], [], []
        self.out_shapes = []
        for alloc in nc.m.functions[0].allocations:
            if not isinstance(alloc, mybir.MemoryLocationSet):
                continue
            name = alloc.memorylocations[0].name
            if alloc.kind == "ExternalInput":
                if name != partition_name:
                    in_names.append(name)
            elif alloc.kind == "ExternalOutput":
                shape = tuple(alloc.tensor_shape)
                dtype = mybir.dt.np(alloc.dtype)
                out_names.append(name)
                out_avals.append(jax.core.ShapedArray(shape, dtype))
                self.out_shapes.append((shape, dtype))
        self.in_names = in_names
        self.out_names = out_names
        bind_names = list(in_names + out_names)
        if partition_name is not None:
            bind_names.append(partition_name)
        bind_names = tuple(bind_names)

        n_in = len(in_names)

        def _body(*args):
            operands = list(args)
            if partition_name is not None:
                operands.append(bass2jax.partition_id_tensor())
            # repeated binds: BassEffect is an ordered effect, so launches
            # serialize and aren't CSE'd despite identical operands
            for _ in range(unroll):
                outs = bass2jax._bass_exec_p.bind(
                    *operands,
                    out_avals=tuple(out_avals),
                    in_names=bind_names,
                    out_names=tuple(out_names),
                    lowering_input_output_aliases=(),
                    sim_require_finite=False,
                    sim_require_nnan=False,
                    nc=nc,
                )
            return tuple(outs)

        from jax.sharding import Mesh, NamedSharding, PartitionSpec
        from jax.experimental.shard_map import shard_map

        devices = jax.devices()[:NCORES]
        self.mesh = Mesh(np.asarray(devices), ("core",))
        self.sharding = NamedSharding(self.mesh, PartitionSpec("core"))
        n_args = len(in_names) + len(out_names)
        self.fn = jax.jit(
            shard_map(
                _body,
                mesh=self.mesh,
                in_specs=(PartitionSpec("core"),) * n_args,
                out_specs=(PartitionSpec("core"),) * len(out_names),
                check_rep=False,
            ),
            keep_unused=True,
        )
        self.args = None

    def set_inputs(self, in_maps):
        import jax

        per_core = [[np.asarray(m[n]) for n in self.in_names] for m in in_maps]
        arrs = [
            np.concatenate([per_core[c][i] for c in range(NCORES)], axis=0)
            for i in range(len(self.in_names))
        ]
        for shape, dtype in self.out_shapes:
            arrs.append(np.zeros((NCORES * shape[0],) + shape[1:], dtype))
        self.args = [jax.device_put(a, self.sharding) for a in arrs]

    def run(self):
        import jax

        outs = self.fn(*self.args)
        jax.block_until_ready(outs)
        return outs


def make_in_maps(feature_emb: np.ndarray, bilinear_W: np.ndarray):
    feature_emb = np.ascontiguousarray(feature_emb, dtype=np.float32)
    bilinear_W = np.ascontiguousarray(bilinear_W, dtype=np.float32)
    assert feature_emb.shape == (B_TOTAL, NF, D)
    assert bilinear_W.shape == (NPAIR, D, D)
    return [
        {
            "feature_emb": feature_emb[c * B_CORE : (c + 1) * B_CORE],
            "bilinear_W": bilinear_W,
        }
        for c in range(NCORES)
    ]


def kernel(feature_emb: np.ndarray, bilinear_W: np.ndarray) -> np.ndarray:
    from concourse.bass_utils import run_bass_kernel_spmd

    in_maps = make_in_maps(feature_emb, bilinear_W)
    nc = _get_nc()
    res = run_bass_kernel_spmd(nc, in_maps, core_ids=list(range(NCORES)))
    return np.concatenate([r["out"] for r in res.results], axis=0)
